# revision 12
# baseline (speedup 1.0000x reference)
"""AdaptiveRankTensorizedLinear (CP, rank 64) forward on 8 TRN2 NeuronCores.

Math: with A = KhatriRao(U1,U2,U3) (4096x64), B = KhatriRao(V1,V2,V3) (4096x64),
    y = (x @ (A * lam)) @ B^T + bias
Data-parallel over the 4096-token batch: each core handles 512 rows of x.
Factors are tiny and replicated; no collectives needed in forward.

Per-core dataflow (all compute on device, bf16 matmuls with f32 accumulate):
  - prologue: replicate U2/U3 across partitions with selection-matrix matmuls
    (S[k,p] one-hot) so A chunks [128k, 64r] build with two elementwise muls;
    B^T (augmented with bias as a rank-65 row) builds from transposed V loads
    with broadcast-AP multiplies (r on partitions).
  - per 128-row m-tile: DMA x f32 -> cast bf16 (GpSimd/ACT) -> transpose
    128x128 chunks on TensorE via identity matmuls (k must land on partitions)
    -> t^T = sum_c A_c^T @ xT_c (PSUM accumulate) -> y = t_aug^T @ BT_aug
    (ones row of t_aug adds bias) -> per-512-col DMA out.
"""

import numpy as np

NCORES = 8
B_TOTAL = 4096
B_SHARD = B_TOTAL // NCORES  # 512
IN = 4096
OUT = 4096
D = 16
R = 64

M_TILE = 128
N_MTILES = B_SHARD // M_TILE  # 4
KCHUNK = 128
N_KCHUNKS = IN // KCHUNK  # 32

_CACHE = {}


def _build_nc():
    from contextlib import ExitStack

    from concourse import bacc, mybir
    import concourse.tile as tile
    from concourse.masks import make_identity

    f32 = mybir.dt.float32
    bf16 = mybir.dt.bfloat16

    nc = bacc.Bacc(None, target_bir_lowering=False)

    x_ext = nc.declare_dram_parameter("x", [B_SHARD, IN], f32, isOutput=False)
    U1_ext = nc.declare_dram_parameter("U1", [D, R], f32, isOutput=False)
    U2_ext = nc.declare_dram_parameter("U2", [D, R], f32, isOutput=False)
    U3_ext = nc.declare_dram_parameter("U3", [D, R], f32, isOutput=False)
    V1_ext = nc.declare_dram_parameter("V1", [D, R], f32, isOutput=False)
    V2_ext = nc.declare_dram_parameter("V2", [D, R], f32, isOutput=False)
    V3_ext = nc.declare_dram_parameter("V3", [D, R], f32, isOutput=False)
    lam_ext = nc.declare_dram_parameter("lam", [R], f32, isOutput=False)
    bias_ext = nc.declare_dram_parameter("bias", [OUT], f32, isOutput=False)
    out_ext = nc.declare_dram_parameter("out", [B_SHARD, OUT], f32, isOutput=True)

    with tile.TileContext(nc) as tc, ExitStack() as ctx:
        const = ctx.enter_context(tc.tile_pool(name="const", bufs=1))
        x_pool = ctx.enter_context(tc.tile_pool(name="x", bufs=3))
        xbf_pool = ctx.enter_context(tc.tile_pool(name="xbf", bufs=2))
        xT_pool = ctx.enter_context(tc.tile_pool(name="xT", bufs=6))
        y_pool = ctx.enter_context(tc.tile_pool(name="y", bufs=6))
        psx_pool = ctx.enter_context(tc.tile_pool(name="psx", bufs=4, space="PSUM"))
        pst_pool = ctx.enter_context(tc.tile_pool(name="pst", bufs=2, space="PSUM"))
        psy_pool = ctx.enter_context(tc.tile_pool(name="psy", bufs=2, space="PSUM"))

        # ------------- prologue ---------------------------------------------
        identity = const.tile([128, 128], bf16)
        make_identity(nc, identity[:])

        # natural tiny loads (contiguous, fast descriptors)
        U2n = const.tile([D, R], f32)
        U3n = const.tile([D, R], f32)
        nc.sync.dma_start(out=U2n[:], in_=U2_ext[:])
        nc.sync.dma_start(out=U3n[:], in_=U3_ext[:])
        # U1 replicated whole to every partition (natural dst; src 0-stride)
        U1rep = const.tile([128, D * R], f32)
        nc.scalar.dma_start(
            out=U1rep[:],
            in_=U1_ext[:].flatten().unsqueeze(0).broadcast_to([128, D * R]),
        )
        # transposed V loads: ViT[r, o] = Vi[o, r]
        V1T = const.tile([R, D], f32)
        V2T = const.tile([R, D], f32)
        V3T = const.tile([R, D], f32)
        nc.scalar.dma_start(out=V1T[:], in_=V1_ext[:].transpose([1, 0]))
        nc.sync.dma_start(out=V2T[:], in_=V2_ext[:].transpose([1, 0]))
        nc.gpsimd.dma_start(out=V3T[:], in_=V3_ext[:].transpose([1, 0]))
        lamT = const.tile([R, 1], f32)
        nc.sync.dma_start(out=lamT[:], in_=lam_ext[:].unsqueeze(1))
        bias_sb = const.tile([1, OUT], f32)
        nc.scalar.dma_start(out=bias_sb[:], in_=bias_ext[:].unsqueeze(0))

        # selection matrices: S3[k, p]=1 iff k==p%16 ; S2h[k, p]=1 iff k==8h+p//16
        # (partition-axis replication = one-hot matmul on PE)
        S3 = const.tile([D, 128], bf16)
        nc.gpsimd.memset(S3[:], 0.0)
        nc.gpsimd.affine_select(
            out=S3[:], in_=S3[:], compare_op=mybir.AluOpType.not_equal,
            fill=1.0, base=0, pattern=[[0, 8], [-1, 16]], channel_multiplier=1,
        )
        S2 = []
        for h in range(2):
            s = const.tile([D, 128], bf16, tag=f"S2_{h}")
            nc.gpsimd.memset(s[:], 0.0)
            nc.gpsimd.affine_select(
                out=s[:], in_=s[:], compare_op=mybir.AluOpType.not_equal,
                fill=1.0, base=-8 * h, pattern=[[-1, 8], [0, 16]],
                channel_multiplier=1,
            )
            S2.append(s)

        # bf16 casts of U2/U3 for the one-hot matmuls
        U2b = const.tile([D, R], bf16)
        U3b = const.tile([D, R], bf16)
        nc.gpsimd.tensor_copy(U2b[:], U2n[:])
        nc.gpsimd.tensor_copy(U3b[:], U3n[:])

        # replicate across partitions: psum rows p get U*[f(p), :]
        ps_rep = psx_pool.tile([128, 4 * R], f32, tag="ps_x")
        nc.tensor.matmul(ps_rep[:, 0:R], S3[:], U3b[:], start=True, stop=True)
        for h in range(2):
            nc.tensor.matmul(
                ps_rep[:, (1 + h) * R : (2 + h) * R], S2[h][:], U2b[:],
                start=True, stop=True,
            )
        U3rep = const.tile([128, R], f32)
        nc.vector.tensor_copy(U3rep[:], ps_rep[:, 0:R])
        # B23[p, 64h + r] = U2[8h + p//16, r] * U3[p%16, r]
        B23 = const.tile([128, 2 * R], f32)
        nc.vector.tensor_mul(
            B23[:].rearrange("p (h r) -> p h r", h=2),
            ps_rep[:, R : 3 * R].rearrange("p (h r) -> p h r", h=2),
            U3rep[:].unsqueeze(1).broadcast_to([128, 2, R]),
        )
        # A chunks: A_sb[p, 64c + r] = U1[c//2, r] * B23[p, 64*(c%2) + r]
        A_sb = const.tile([128, N_KCHUNKS * R], bf16)
        nc.vector.tensor_mul(
            A_sb[:].rearrange("p (i g r) -> p i g r", i=16, g=2),
            U1rep[:].rearrange("p (i r) -> p i r", i=16)
            .unsqueeze(2)
            .broadcast_to([128, 16, 2, R]),
            B23[:].rearrange("p (g r) -> p g r", g=2)
            .unsqueeze(1)
            .broadcast_to([128, 16, 2, R]),
        )

        # BT_aug rows 0..63: lam[r]*V1[o1,r]*V2[o2,r]*V3[o3,r]; row 64: bias
        V1Ts = const.tile([R, D], f32)
        nc.gpsimd.tensor_mul(V1Ts[:], V1T[:], lamT[:].broadcast_to([R, D]))
        W12v = const.tile([R, D * D], f32)
        nc.gpsimd.tensor_mul(
            W12v[:].rearrange("p (a b) -> p a b", a=16),
            V1Ts[:].unsqueeze(2).broadcast_to([R, D, D]),
            V2T[:].unsqueeze(1).broadcast_to([R, D, D]),
        )
        BT_aug = const.tile([R + 1, OUT], bf16)
        for half in range(2):
            nc.vector.tensor_mul(
                BT_aug[0:R, half * (OUT // 2) : (half + 1) * (OUT // 2)].rearrange(
                    "p (w o) -> p w o", o=16
                ),
                W12v[:, half * (D * D // 2) : (half + 1) * (D * D // 2)]
                .unsqueeze(2)
                .broadcast_to([R, D * D // 2, D]),
                V3T[:].unsqueeze(1).broadcast_to([R, D * D // 2, D]),
            )
        nc.scalar.copy(BT_aug[R : R + 1, :], bias_sb[:])

        # two persistent t_aug tiles (double buffer), ones row preset
        t_aug = []
        for i in range(2):
            t = const.tile([R + 1, M_TILE], bf16, tag=f"t_aug{i}")
            nc.gpsimd.memset(t[R : R + 1, :], 1.0)
            t_aug.append(t)

        # ---------------- main loop ------------------------------------------
        for m in range(N_MTILES):
            x_sb = x_pool.tile([M_TILE, IN], f32)
            nc.sync.dma_start(
                out=x_sb[:, 0 : IN // 2],
                in_=x_ext[m * M_TILE : (m + 1) * M_TILE, 0 : IN // 2],
            )
            nc.sync.dma_start(
                out=x_sb[:, IN // 2 :],
                in_=x_ext[m * M_TILE : (m + 1) * M_TILE, IN // 2 :],
            )
            x_bf = xbf_pool.tile([M_TILE, IN], bf16)

            ps_t = pst_pool.tile([R, M_TILE], f32)
            for g in range(N_KCHUNKS // 4):
                lo, hi = g * 4 * KCHUNK, (g + 1) * 4 * KCHUNK
                # cast this 512-col group to bf16 (GpSimd mostly: it is idle)
                if g % 4 == 3:
                    nc.scalar.copy(x_bf[:, lo:hi], x_sb[:, lo:hi])
                else:
                    nc.gpsimd.tensor_copy(x_bf[:, lo:hi], x_sb[:, lo:hi])
                ps_x = psx_pool.tile([128, 4 * KCHUNK], f32)
                for j in range(4):
                    c = 4 * g + j
                    nc.tensor.matmul(
                        ps_x[:, j * KCHUNK : (j + 1) * KCHUNK],
                        x_bf[:, c * KCHUNK : (c + 1) * KCHUNK],
                        identity[:],
                        start=True,
                        stop=True,
                    )
                xT_sb = xT_pool.tile([128, 4 * KCHUNK], bf16)
                if g % 2 == 0:
                    nc.scalar.copy(xT_sb[:], ps_x[:])
                else:
                    nc.vector.tensor_copy(xT_sb[:], ps_x[:])
                for j in range(4):
                    c = 4 * g + j
                    nc.tensor.matmul(
                        ps_t[:],
                        A_sb[:, c * R : (c + 1) * R],
                        xT_sb[:, j * KCHUNK : (j + 1) * KCHUNK],
                        start=(c == 0),
                        stop=(c == N_KCHUNKS - 1),
                    )

            tt = t_aug[m % 2]
            nc.vector.tensor_copy(tt[0:R, :], ps_t[:])

            for n in range(8):
                ps_y = psy_pool.tile([M_TILE, 512], f32)
                nc.tensor.matmul(
                    ps_y[:],
                    tt[:],
                    BT_aug[:, n * 512 : (n + 1) * 512],
                    start=True,
                    stop=True,
                )
                y_sb = y_pool.tile([M_TILE, 512], f32)
                if n % 2 == 0:
                    nc.vector.tensor_copy(y_sb[:], ps_y[:])
                else:
                    nc.scalar.copy(y_sb[:], ps_y[:])
                nc.sync.dma_start(
                    out=out_ext[
                        m * M_TILE : (m + 1) * M_TILE, n * 512 : (n + 1) * 512
                    ],
                    in_=y_sb[:],
                )

    nc.compile()
    return nc


def _get_nc():
    if "nc" not in _CACHE:
        _CACHE["nc"] = _build_nc()
    return _CACHE["nc"]


def kernel(x, U1, U2, U3, V1, V2, V3, lam, bias):
    from concourse.bass_utils import run_bass_kernel_spmd

    nc = _get_nc()

    x = np.ascontiguousarray(np.asarray(x, dtype=np.float32))
    small = {
        "U1": U1, "U2": U2, "U3": U3,
        "V1": V1, "V2": V2, "V3": V3,
        "lam": lam, "bias": bias,
    }
    small = {
        k: np.ascontiguousarray(np.asarray(v, dtype=np.float32))
        for k, v in small.items()
    }

    in_maps = [
        {"x": x[i * B_SHARD : (i + 1) * B_SHARD], **small} for i in range(NCORES)
    ]
    res = run_bass_kernel_spmd(nc, in_maps, core_ids=list(range(NCORES)))
    _CACHE["last_results"] = res
    out = np.concatenate(
        [np.asarray(res.results[i]["out"]) for i in range(NCORES)], axis=0
    )
    return out.astype(np.float32)


def last_exec_time_ns():
    res = _CACHE.get("last_results")
    return None if res is None else res.exec_time_ns


# revision 14
# speedup vs baseline: 1.3421x; 1.3421x over previous
"""AdaptiveRankTensorizedLinear (CP, rank 64) forward on 8 TRN2 NeuronCores.

Math: with A = KhatriRao(U1,U2,U3) (4096x64), B = KhatriRao(V1,V2,V3) (4096x64),
    y = (x @ (A * lam)) @ B^T + bias
Data-parallel over the 4096-token batch: each core handles 512 rows of x.
Factors are tiny and replicated; no collectives needed in forward.

Per-core dataflow (all compute on device, bf16 matmuls with f32 accumulate):
  - prologue: replicate U2/U3 across partitions with selection-matrix matmuls
    (S[k,p] one-hot) so A chunks [128k, 64r] build with two elementwise muls;
    B^T (augmented with bias as a rank-65 row) builds from transposed V loads
    with broadcast-AP multiplies (r on partitions).
  - per 128-row m-tile: DMA x f32 -> cast bf16 (GpSimd/ACT) -> transpose
    128x128 chunks on TensorE via identity matmuls (k must land on partitions)
    -> t^T = sum_c A_c^T @ xT_c (PSUM accumulate) -> y = t_aug^T @ BT_aug
    (ones row of t_aug adds bias) -> per-512-col DMA out.
"""

import numpy as np

NCORES = 8
B_TOTAL = 4096
B_SHARD = B_TOTAL // NCORES  # 512
IN = 4096
OUT = 4096
D = 16
R = 64

M_TILE = 128
N_MTILES = B_SHARD // M_TILE  # 4
KCHUNK = 128
N_KCHUNKS = IN // KCHUNK  # 32

_CACHE = {}


def _build_nc():
    from contextlib import ExitStack

    from concourse import bacc, mybir
    import concourse.tile as tile
    from concourse.masks import make_identity

    f32 = mybir.dt.float32
    bf16 = mybir.dt.bfloat16

    nc = bacc.Bacc(None, target_bir_lowering=False)

    x_ext = nc.declare_dram_parameter("x", [B_SHARD, IN], f32, isOutput=False)
    U1_ext = nc.declare_dram_parameter("U1", [D, R], f32, isOutput=False)
    U2_ext = nc.declare_dram_parameter("U2", [D, R], f32, isOutput=False)
    U3_ext = nc.declare_dram_parameter("U3", [D, R], f32, isOutput=False)
    V1_ext = nc.declare_dram_parameter("V1", [D, R], f32, isOutput=False)
    V2_ext = nc.declare_dram_parameter("V2", [D, R], f32, isOutput=False)
    V3_ext = nc.declare_dram_parameter("V3", [D, R], f32, isOutput=False)
    lam_ext = nc.declare_dram_parameter("lam", [R], f32, isOutput=False)
    bias_ext = nc.declare_dram_parameter("bias", [OUT], f32, isOutput=False)
    out_ext = nc.declare_dram_parameter("out", [B_SHARD, OUT], f32, isOutput=True)

    with tile.TileContext(nc) as tc, ExitStack() as ctx:
        const = ctx.enter_context(tc.tile_pool(name="const", bufs=1))
        x_pool = ctx.enter_context(tc.tile_pool(name="x", bufs=3))
        xbf_pool = ctx.enter_context(tc.tile_pool(name="xbf", bufs=2))
        xT_pool = ctx.enter_context(tc.tile_pool(name="xT", bufs=6))
        y_pool = ctx.enter_context(tc.tile_pool(name="y", bufs=6))
        psx_pool = ctx.enter_context(tc.tile_pool(name="psx", bufs=4, space="PSUM"))
        pst_pool = ctx.enter_context(tc.tile_pool(name="pst", bufs=2, space="PSUM"))
        psy_pool = ctx.enter_context(tc.tile_pool(name="psy", bufs=2, space="PSUM"))

        # ------------- prologue ---------------------------------------------
        identity = const.tile([128, 128], bf16)
        make_identity(nc, identity[:])

        # natural tiny loads (contiguous, fast descriptors)
        U2n = const.tile([D, R], f32)
        U3n = const.tile([D, R], f32)
        nc.sync.dma_start(out=U2n[:], in_=U2_ext[:])
        nc.sync.dma_start(out=U3n[:], in_=U3_ext[:])
        # U1 replicated whole to every partition (natural dst; src 0-stride)
        U1rep = const.tile([128, D * R], f32)
        nc.scalar.dma_start(
            out=U1rep[:],
            in_=U1_ext[:].flatten().unsqueeze(0).broadcast_to([128, D * R]),
        )
        # transposed V loads: ViT[r, o] = Vi[o, r]
        V1T = const.tile([R, D], f32)
        V2T = const.tile([R, D], f32)
        V3T = const.tile([R, D], f32)
        nc.scalar.dma_start(out=V1T[:], in_=V1_ext[:].transpose([1, 0]))
        nc.sync.dma_start(out=V2T[:], in_=V2_ext[:].transpose([1, 0]))
        nc.gpsimd.dma_start(out=V3T[:], in_=V3_ext[:].transpose([1, 0]))
        lamT = const.tile([R, 1], f32)
        nc.sync.dma_start(out=lamT[:], in_=lam_ext[:].unsqueeze(1))
        bias_sb = const.tile([1, OUT], f32)
        nc.scalar.dma_start(out=bias_sb[:], in_=bias_ext[:].unsqueeze(0))

        # selection matrices: S3[k, p]=1 iff k==p%16 ; S2h[k, p]=1 iff k==8h+p//16
        # (partition-axis replication = one-hot matmul on PE)
        S3 = const.tile([D, 128], bf16)
        nc.gpsimd.memset(S3[:], 0.0)
        nc.gpsimd.affine_select(
            out=S3[:], in_=S3[:], compare_op=mybir.AluOpType.not_equal,
            fill=1.0, base=0, pattern=[[0, 8], [-1, 16]], channel_multiplier=1,
        )
        S2 = []
        for h in range(2):
            s = const.tile([D, 128], bf16, tag=f"S2_{h}")
            nc.gpsimd.memset(s[:], 0.0)
            nc.gpsimd.affine_select(
                out=s[:], in_=s[:], compare_op=mybir.AluOpType.not_equal,
                fill=1.0, base=-8 * h, pattern=[[-1, 8], [0, 16]],
                channel_multiplier=1,
            )
            S2.append(s)

        # bf16 casts of U2/U3 for the one-hot matmuls
        U2b = const.tile([D, R], bf16)
        U3b = const.tile([D, R], bf16)
        nc.gpsimd.tensor_copy(U2b[:], U2n[:])
        nc.gpsimd.tensor_copy(U3b[:], U3n[:])

        # replicate across partitions: psum rows p get U*[f(p), :]
        ps_rep = psx_pool.tile([128, 4 * R], f32, tag="ps_x")
        nc.tensor.matmul(ps_rep[:, 0:R], S3[:], U3b[:], start=True, stop=True)
        for h in range(2):
            nc.tensor.matmul(
                ps_rep[:, (1 + h) * R : (2 + h) * R], S2[h][:], U2b[:],
                start=True, stop=True,
            )
        U3rep = const.tile([128, R], f32)
        nc.vector.tensor_copy(U3rep[:], ps_rep[:, 0:R])
        # B23[p, 64h + r] = U2[8h + p//16, r] * U3[p%16, r]
        B23 = const.tile([128, 2 * R], f32)
        nc.vector.tensor_mul(
            B23[:].rearrange("p (h r) -> p h r", h=2),
            ps_rep[:, R : 3 * R].rearrange("p (h r) -> p h r", h=2),
            U3rep[:].unsqueeze(1).broadcast_to([128, 2, R]),
        )
        # A chunks: A_sb[p, 64c + r] = U1[c//2, r] * B23[p, 64*(c%2) + r]
        A_sb = const.tile([128, N_KCHUNKS * R], bf16)
        nc.vector.tensor_mul(
            A_sb[:].rearrange("p (i g r) -> p i g r", i=16, g=2),
            U1rep[:].rearrange("p (i r) -> p i r", i=16)
            .unsqueeze(2)
            .broadcast_to([128, 16, 2, R]),
            B23[:].rearrange("p (g r) -> p g r", g=2)
            .unsqueeze(1)
            .broadcast_to([128, 16, 2, R]),
        )

        # BT_aug rows 0..63: lam[r]*V1[o1,r]*V2[o2,r]*V3[o3,r]; row 64: bias
        V1Ts = const.tile([R, D], f32)
        nc.gpsimd.tensor_mul(V1Ts[:], V1T[:], lamT[:].broadcast_to([R, D]))
        W12v = const.tile([R, D * D], f32)
        nc.gpsimd.tensor_mul(
            W12v[:].rearrange("p (a b) -> p a b", a=16),
            V1Ts[:].unsqueeze(2).broadcast_to([R, D, D]),
            V2T[:].unsqueeze(1).broadcast_to([R, D, D]),
        )
        BT_aug = const.tile([R + 1, OUT], bf16)
        for half in range(2):
            nc.vector.tensor_mul(
                BT_aug[0:R, half * (OUT // 2) : (half + 1) * (OUT // 2)].rearrange(
                    "p (w o) -> p w o", o=16
                ),
                W12v[:, half * (D * D // 2) : (half + 1) * (D * D // 2)]
                .unsqueeze(2)
                .broadcast_to([R, D * D // 2, D]),
                V3T[:].unsqueeze(1).broadcast_to([R, D * D // 2, D]),
            )
        nc.scalar.copy(BT_aug[R : R + 1, :], bias_sb[:])

        # two persistent t_aug tiles (double buffer), ones row preset
        t_aug = []
        for i in range(2):
            t = const.tile([R + 1, M_TILE], bf16, tag=f"t_aug{i}")
            nc.gpsimd.memset(t[R : R + 1, :], 1.0)
            t_aug.append(t)

        # ---------------- main loop ------------------------------------------
        for m in range(N_MTILES):
            x_sb = x_pool.tile([M_TILE, IN], f32)
            nc.sync.dma_start(
                out=x_sb[:, 0 : IN // 2],
                in_=x_ext[m * M_TILE : (m + 1) * M_TILE, 0 : IN // 2],
            )
            nc.sync.dma_start(
                out=x_sb[:, IN // 2 :],
                in_=x_ext[m * M_TILE : (m + 1) * M_TILE, IN // 2 :],
            )
            x_bf = xbf_pool.tile([M_TILE, IN], bf16)

            ps_t = pst_pool.tile([R, M_TILE], f32)
            for g in range(N_KCHUNKS // 4):
                lo, hi = g * 4 * KCHUNK, (g + 1) * 4 * KCHUNK
                # cast this 512-col group to bf16
                if g % 2 == 0:
                    nc.scalar.copy(x_bf[:, lo:hi], x_sb[:, lo:hi])
                else:
                    nc.vector.tensor_copy(x_bf[:, lo:hi], x_sb[:, lo:hi])
                ps_x = psx_pool.tile([128, 4 * KCHUNK], f32)
                for j in range(4):
                    c = 4 * g + j
                    nc.tensor.matmul(
                        ps_x[:, j * KCHUNK : (j + 1) * KCHUNK],
                        x_bf[:, c * KCHUNK : (c + 1) * KCHUNK],
                        identity[:],
                        start=True,
                        stop=True,
                    )
                xT_sb = xT_pool.tile([128, 4 * KCHUNK], bf16)
                if g % 2 == 0:
                    nc.vector.tensor_copy(xT_sb[:], ps_x[:])
                else:
                    nc.scalar.copy(xT_sb[:], ps_x[:])
                for j in range(4):
                    c = 4 * g + j
                    nc.tensor.matmul(
                        ps_t[:],
                        A_sb[:, c * R : (c + 1) * R],
                        xT_sb[:, j * KCHUNK : (j + 1) * KCHUNK],
                        start=(c == 0),
                        stop=(c == N_KCHUNKS - 1),
                    )

            tt = t_aug[m % 2]
            nc.vector.tensor_copy(tt[0:R, :], ps_t[:])

            for n in range(8):
                ps_y = psy_pool.tile([M_TILE, 512], f32)
                nc.tensor.matmul(
                    ps_y[:],
                    tt[:],
                    BT_aug[:, n * 512 : (n + 1) * 512],
                    start=True,
                    stop=True,
                )
                y_sb = y_pool.tile([M_TILE, 512], f32)
                if n % 2 == 0:
                    nc.vector.tensor_copy(y_sb[:], ps_y[:])
                else:
                    nc.scalar.copy(y_sb[:], ps_y[:])
                nc.sync.dma_start(
                    out=out_ext[
                        m * M_TILE : (m + 1) * M_TILE, n * 512 : (n + 1) * 512
                    ],
                    in_=y_sb[:],
                )

    nc.compile()
    return nc


def _get_nc():
    if "nc" not in _CACHE:
        _CACHE["nc"] = _build_nc()
    return _CACHE["nc"]


def kernel(x, U1, U2, U3, V1, V2, V3, lam, bias):
    from concourse.bass_utils import run_bass_kernel_spmd

    nc = _get_nc()

    x = np.ascontiguousarray(np.asarray(x, dtype=np.float32))
    small = {
        "U1": U1, "U2": U2, "U3": U3,
        "V1": V1, "V2": V2, "V3": V3,
        "lam": lam, "bias": bias,
    }
    small = {
        k: np.ascontiguousarray(np.asarray(v, dtype=np.float32))
        for k, v in small.items()
    }

    in_maps = [
        {"x": x[i * B_SHARD : (i + 1) * B_SHARD], **small} for i in range(NCORES)
    ]
    res = run_bass_kernel_spmd(nc, in_maps, core_ids=list(range(NCORES)))
    _CACHE["last_results"] = res
    out = np.concatenate(
        [np.asarray(res.results[i]["out"]) for i in range(NCORES)], axis=0
    )
    return out.astype(np.float32)


def last_exec_time_ns():
    res = _CACHE.get("last_results")
    return None if res is None else res.exec_time_ns


# revision 15
# speedup vs baseline: 1.4892x; 1.1096x over previous
"""AdaptiveRankTensorizedLinear (CP, rank 64) forward on 8 TRN2 NeuronCores.

Math: with A = KhatriRao(U1,U2,U3) (4096x64), B = KhatriRao(V1,V2,V3) (4096x64),
    y = (x @ (A * lam)) @ B^T + bias
Data-parallel over the 4096-token batch: each core handles 512 rows of x.
Factors are tiny and replicated; no collectives needed in forward.

Per-core dataflow (all compute on device, bf16 matmuls with f32 accumulate):
  - prologue: replicate U2/U3 across partitions with selection-matrix matmuls
    (S[k,p] one-hot) so A chunks [128k, 64r] build with two elementwise muls;
    B^T (augmented with bias as a rank-65 row) builds from transposed V loads
    with broadcast-AP multiplies (r on partitions).
  - per 128-row m-tile: DMA x f32 -> cast bf16 -> transpose 128x128 chunks on
    TensorE via identity matmuls (k must land on partitions) -> t^T =
    sum_c A_c^T @ xT_c (PSUM accumulate) -> y = t_aug^T @ BT_aug (ones row of
    t_aug adds bias) -> per-512-col DMA out.
"""

import numpy as np

NCORES = 8
B_TOTAL = 4096
B_SHARD = B_TOTAL // NCORES  # 512
IN = 4096
OUT = 4096
D = 16
R = 64

M_TILE = 128
N_MTILES = B_SHARD // M_TILE  # 4
KCHUNK = 128
N_KCHUNKS = IN // KCHUNK  # 32
GCHUNKS = 8  # k-chunks per cast/copy group (1024 cols)
N_GROUPS = N_KCHUNKS // GCHUNKS  # 4

_CACHE = {}


def _build_nc():
    from contextlib import ExitStack

    from concourse import bacc, mybir
    import concourse.tile as tile
    from concourse.masks import make_identity

    f32 = mybir.dt.float32
    bf16 = mybir.dt.bfloat16

    nc = bacc.Bacc(None, target_bir_lowering=False)

    x_ext = nc.declare_dram_parameter("x", [B_SHARD, IN], f32, isOutput=False)
    U1_ext = nc.declare_dram_parameter("U1", [D, R], f32, isOutput=False)
    U2_ext = nc.declare_dram_parameter("U2", [D, R], f32, isOutput=False)
    U3_ext = nc.declare_dram_parameter("U3", [D, R], f32, isOutput=False)
    V1_ext = nc.declare_dram_parameter("V1", [D, R], f32, isOutput=False)
    V2_ext = nc.declare_dram_parameter("V2", [D, R], f32, isOutput=False)
    V3_ext = nc.declare_dram_parameter("V3", [D, R], f32, isOutput=False)
    lam_ext = nc.declare_dram_parameter("lam", [R], f32, isOutput=False)
    bias_ext = nc.declare_dram_parameter("bias", [OUT], f32, isOutput=False)
    out_ext = nc.declare_dram_parameter("out", [B_SHARD, OUT], f32, isOutput=True)

    with tile.TileContext(nc) as tc, ExitStack() as ctx:
        const = ctx.enter_context(tc.tile_pool(name="const", bufs=1))
        x_pool = ctx.enter_context(tc.tile_pool(name="x", bufs=3))
        xbf_pool = ctx.enter_context(tc.tile_pool(name="xbf", bufs=3))
        xT_pool = ctx.enter_context(tc.tile_pool(name="xT", bufs=3))
        y_pool = ctx.enter_context(tc.tile_pool(name="y", bufs=6))
        psx_pool = ctx.enter_context(tc.tile_pool(name="psx", bufs=2, space="PSUM"))
        pst_pool = ctx.enter_context(tc.tile_pool(name="pst", bufs=1, space="PSUM"))
        psy_pool = ctx.enter_context(tc.tile_pool(name="psy", bufs=3, space="PSUM"))

        # ------------- prologue ---------------------------------------------
        identity = const.tile([128, 128], bf16)
        make_identity(nc, identity[:])

        # selection matrices first on gpsimd (no input deps):
        # S3[k, p]=1 iff k==p%16 ; S2h[k, p]=1 iff k==8h+p//16
        S3 = const.tile([D, 128], bf16)
        nc.gpsimd.memset(S3[:], 0.0)
        nc.gpsimd.affine_select(
            out=S3[:], in_=S3[:], compare_op=mybir.AluOpType.not_equal,
            fill=1.0, base=0, pattern=[[0, 8], [-1, 16]], channel_multiplier=1,
        )
        S2 = []
        for h in range(2):
            s = const.tile([D, 128], bf16, tag=f"S2_{h}")
            nc.gpsimd.memset(s[:], 0.0)
            nc.gpsimd.affine_select(
                out=s[:], in_=s[:], compare_op=mybir.AluOpType.not_equal,
                fill=1.0, base=-8 * h, pattern=[[-1, 8], [0, 16]],
                channel_multiplier=1,
            )
            S2.append(s)
        t_aug = []
        for i in range(2):
            t = const.tile([R + 1, M_TILE], bf16, tag=f"t_aug{i}")
            nc.gpsimd.memset(t[R : R + 1, :], 1.0)
            t_aug.append(t)

        # tiny loads
        U2n = const.tile([D, R], f32)
        U3n = const.tile([D, R], f32)
        nc.sync.dma_start(out=U2n[:], in_=U2_ext[:])
        nc.sync.dma_start(out=U3n[:], in_=U3_ext[:])
        U1rep = const.tile([128, D * R], f32)
        nc.scalar.dma_start(
            out=U1rep[:],
            in_=U1_ext[:].flatten().unsqueeze(0).broadcast_to([128, D * R]),
        )
        V1T = const.tile([R, D], f32)
        V2T = const.tile([R, D], f32)
        V3T = const.tile([R, D], f32)
        nc.scalar.dma_start(out=V1T[:], in_=V1_ext[:].transpose([1, 0]))
        nc.sync.dma_start(out=V2T[:], in_=V2_ext[:].transpose([1, 0]))
        nc.sync.dma_start(out=V3T[:], in_=V3_ext[:].transpose([1, 0]))
        lamT = const.tile([R, 1], f32)
        nc.sync.dma_start(out=lamT[:], in_=lam_ext[:].unsqueeze(1))
        bias_sb = const.tile([1, OUT], f32)
        nc.scalar.dma_start(out=bias_sb[:], in_=bias_ext[:].unsqueeze(0))

        # bf16 casts of U2/U3 for the one-hot matmuls
        U2b = const.tile([D, R], bf16)
        U3b = const.tile([D, R], bf16)
        nc.vector.tensor_copy(U2b[:], U2n[:])
        nc.vector.tensor_copy(U3b[:], U3n[:])

        # replicate across partitions: psum rows p get U*[f(p), :]
        ps_rep = psx_pool.tile([128, 4 * R], f32, tag="ps_x")
        nc.tensor.matmul(ps_rep[:, 0:R], S3[:], U3b[:], start=True, stop=True)
        for h in range(2):
            nc.tensor.matmul(
                ps_rep[:, (1 + h) * R : (2 + h) * R], S2[h][:], U2b[:],
                start=True, stop=True,
            )
        U3rep = const.tile([128, R], f32)
        nc.vector.tensor_copy(U3rep[:], ps_rep[:, 0:R])
        # B23[p, 64h + r] = U2[8h + p//16, r] * U3[p%16, r]
        B23 = const.tile([128, 2 * R], f32)
        nc.vector.tensor_mul(
            B23[:].rearrange("p (h r) -> p h r", h=2),
            ps_rep[:, R : 3 * R].rearrange("p (h r) -> p h r", h=2),
            U3rep[:].unsqueeze(1).broadcast_to([128, 2, R]),
        )
        # A chunks: A_sb[p, 64c + r] = U1[c//2, r] * B23[p, 64*(c%2) + r]
        A_sb = const.tile([128, N_KCHUNKS * R], bf16)
        nc.vector.tensor_mul(
            A_sb[:].rearrange("p (i g r) -> p i g r", i=16, g=2),
            U1rep[:].rearrange("p (i r) -> p i r", i=16)
            .unsqueeze(2)
            .broadcast_to([128, 16, 2, R]),
            B23[:].rearrange("p (g r) -> p g r", g=2)
            .unsqueeze(1)
            .broadcast_to([128, 16, 2, R]),
        )

        # BT_aug rows 0..63: lam[r]*V1[o1,r]*V2[o2,r]*V3[o3,r]; row 64: bias
        V1Ts = const.tile([R, D], f32)
        nc.gpsimd.tensor_mul(V1Ts[:], V1T[:], lamT[:].broadcast_to([R, D]))
        W12v = const.tile([R, D * D], f32)
        nc.gpsimd.tensor_mul(
            W12v[:].rearrange("p (a b) -> p a b", a=16),
            V1Ts[:].unsqueeze(2).broadcast_to([R, D, D]),
            V2T[:].unsqueeze(1).broadcast_to([R, D, D]),
        )
        BT_aug = const.tile([R + 1, OUT], bf16)
        # first half gpsimd, second half DVE (after the A chain)
        nc.gpsimd.tensor_mul(
            BT_aug[0:R, 0 : OUT // 2].rearrange("p (w o) -> p w o", o=16),
            W12v[:, 0 : D * D // 2].unsqueeze(2).broadcast_to([R, D * D // 2, D]),
            V3T[:].unsqueeze(1).broadcast_to([R, D * D // 2, D]),
        )
        nc.vector.tensor_mul(
            BT_aug[0:R, OUT // 2 :].rearrange("p (w o) -> p w o", o=16),
            W12v[:, D * D // 2 :].unsqueeze(2).broadcast_to([R, D * D // 2, D]),
            V3T[:].unsqueeze(1).broadcast_to([R, D * D // 2, D]),
        )
        nc.scalar.copy(BT_aug[R : R + 1, :], bias_sb[:])

        # single persistent accumulation bank: cols (m%2)*128 per m-tile
        ps_t_all = pst_pool.tile([R, 2 * M_TILE], f32)

        # ---------------- main loop ------------------------------------------
        for m in range(N_MTILES):
            x_sb = x_pool.tile([M_TILE, IN], f32)
            nc.sync.dma_start(
                out=x_sb[:, 0 : IN // 2],
                in_=x_ext[m * M_TILE : (m + 1) * M_TILE, 0 : IN // 2],
            )
            nc.sync.dma_start(
                out=x_sb[:, IN // 2 :],
                in_=x_ext[m * M_TILE : (m + 1) * M_TILE, IN // 2 :],
            )
            x_bf = xbf_pool.tile([M_TILE, IN], bf16)

            ps_t = ps_t_all[:, m % 2 * M_TILE : (m % 2 + 1) * M_TILE]
            for g in range(N_GROUPS):
                lo, hi = g * GCHUNKS * KCHUNK, (g + 1) * GCHUNKS * KCHUNK
                if g % 2 == 0:
                    nc.scalar.copy(x_bf[:, lo:hi], x_sb[:, lo:hi])
                else:
                    nc.vector.tensor_copy(x_bf[:, lo:hi], x_sb[:, lo:hi])
                ps_x = psx_pool.tile([128, GCHUNKS * KCHUNK], f32, tag="ps_x")
                for j in range(GCHUNKS):
                    c = GCHUNKS * g + j
                    nc.tensor.matmul(
                        ps_x[:, j * KCHUNK : (j + 1) * KCHUNK],
                        x_bf[:, c * KCHUNK : (c + 1) * KCHUNK],
                        identity[:],
                        start=True,
                        stop=True,
                    )
                xT_sb = xT_pool.tile([128, GCHUNKS * KCHUNK], bf16)
                if g % 2 == 0:
                    nc.vector.tensor_copy(xT_sb[:], ps_x[:])
                else:
                    nc.scalar.copy(xT_sb[:], ps_x[:])
                for j in range(GCHUNKS):
                    c = GCHUNKS * g + j
                    nc.tensor.matmul(
                        ps_t,
                        A_sb[:, c * R : (c + 1) * R],
                        xT_sb[:, j * KCHUNK : (j + 1) * KCHUNK],
                        start=(c == 0),
                        stop=(c == N_KCHUNKS - 1),
                    )

            tt = t_aug[m % 2]
            nc.vector.tensor_copy(tt[0:R, :], ps_t)

            for n in range(8):
                ps_y = psy_pool.tile([M_TILE, 512], f32)
                nc.tensor.matmul(
                    ps_y[:],
                    tt[:],
                    BT_aug[:, n * 512 : (n + 1) * 512],
                    start=True,
                    stop=True,
                )
                y_sb = y_pool.tile([M_TILE, 512], f32)
                if n % 2 == 0:
                    nc.vector.tensor_copy(y_sb[:], ps_y[:])
                else:
                    nc.scalar.copy(y_sb[:], ps_y[:])
                nc.sync.dma_start(
                    out=out_ext[
                        m * M_TILE : (m + 1) * M_TILE, n * 512 : (n + 1) * 512
                    ],
                    in_=y_sb[:],
                )

    nc.compile()
    return nc


def _get_nc():
    if "nc" not in _CACHE:
        _CACHE["nc"] = _build_nc()
    return _CACHE["nc"]


def kernel(x, U1, U2, U3, V1, V2, V3, lam, bias):
    from concourse.bass_utils import run_bass_kernel_spmd

    nc = _get_nc()

    x = np.ascontiguousarray(np.asarray(x, dtype=np.float32))
    small = {
        "U1": U1, "U2": U2, "U3": U3,
        "V1": V1, "V2": V2, "V3": V3,
        "lam": lam, "bias": bias,
    }
    small = {
        k: np.ascontiguousarray(np.asarray(v, dtype=np.float32))
        for k, v in small.items()
    }

    in_maps = [
        {"x": x[i * B_SHARD : (i + 1) * B_SHARD], **small} for i in range(NCORES)
    ]
    res = run_bass_kernel_spmd(nc, in_maps, core_ids=list(range(NCORES)))
    _CACHE["last_results"] = res
    out = np.concatenate(
        [np.asarray(res.results[i]["out"]) for i in range(NCORES)], axis=0
    )
    return out.astype(np.float32)


def last_exec_time_ns():
    res = _CACHE.get("last_results")
    return None if res is None else res.exec_time_ns


# revision 21
# speedup vs baseline: 1.5373x; 1.0323x over previous
"""AdaptiveRankTensorizedLinear (CP, rank 64) forward on 8 TRN2 NeuronCores.

Math: with A = KhatriRao(U1,U2,U3) (4096x64), B = KhatriRao(V1,V2,V3) (4096x64),
    y = (x @ (A * lam)) @ B^T + bias
Data-parallel over the 4096-token batch: each core handles 512 rows of x.
Factors are tiny and replicated; no collectives needed in forward.

Per-core dataflow (all compute on device, bf16 matmuls with f32 accumulate):
  - prologue: replicate U2/U3 across partitions with selection-matrix matmuls
    (S[k,p] one-hot) so A chunks [128k, 64r] build with two elementwise muls;
    B^T (augmented with bias as a rank-65 row) builds from transposed V loads
    with broadcast-AP multiplies (r on partitions).
  - per 128-row m-tile: DMA x f32 -> cast bf16 -> transpose 128x128 chunks on
    TensorE via identity matmuls (k must land on partitions) -> t^T =
    sum_c A_c^T @ xT_c (PSUM accumulate) -> y = t_aug^T @ BT_aug (ones row of
    t_aug adds bias) -> per-512-col DMA out.
"""

import numpy as np

NCORES = 8
B_TOTAL = 4096
B_SHARD = B_TOTAL // NCORES  # 512
IN = 4096
OUT = 4096
D = 16
R = 64

M_TILE = 128
N_MTILES = B_SHARD // M_TILE  # 4
KCHUNK = 128
N_KCHUNKS = IN // KCHUNK  # 32
GCHUNKS = 8  # k-chunks per cast/copy group (1024 cols)
N_GROUPS = N_KCHUNKS // GCHUNKS  # 4

_CACHE = {}


def _build_nc():
    from contextlib import ExitStack

    from concourse import bacc, mybir
    import concourse.tile as tile
    from concourse.masks import make_identity

    f32 = mybir.dt.float32
    bf16 = mybir.dt.bfloat16

    nc = bacc.Bacc(None, target_bir_lowering=False)

    x_ext = nc.declare_dram_parameter("x", [B_SHARD, IN], f32, isOutput=False)
    U1_ext = nc.declare_dram_parameter("U1", [D, R], f32, isOutput=False)
    U2_ext = nc.declare_dram_parameter("U2", [D, R], f32, isOutput=False)
    U3_ext = nc.declare_dram_parameter("U3", [D, R], f32, isOutput=False)
    V1_ext = nc.declare_dram_parameter("V1", [D, R], f32, isOutput=False)
    V2_ext = nc.declare_dram_parameter("V2", [D, R], f32, isOutput=False)
    V3_ext = nc.declare_dram_parameter("V3", [D, R], f32, isOutput=False)
    lam_ext = nc.declare_dram_parameter("lam", [R], f32, isOutput=False)
    bias_ext = nc.declare_dram_parameter("bias", [OUT], f32, isOutput=False)
    out_ext = nc.declare_dram_parameter("out", [B_SHARD, OUT], f32, isOutput=True)

    with tile.TileContext(nc) as tc, ExitStack() as ctx:
        const = ctx.enter_context(tc.tile_pool(name="const", bufs=1))
        x_pool = ctx.enter_context(tc.tile_pool(name="x", bufs=4))
        xbf_pool = ctx.enter_context(tc.tile_pool(name="xbf", bufs=3))
        xT_pool = ctx.enter_context(tc.tile_pool(name="xT", bufs=3))
        y_pool = ctx.enter_context(tc.tile_pool(name="y", bufs=6))
        psx_pool = ctx.enter_context(tc.tile_pool(name="psx", bufs=2, space="PSUM"))
        pst_pool = ctx.enter_context(tc.tile_pool(name="pst", bufs=1, space="PSUM"))
        psy_pool = ctx.enter_context(tc.tile_pool(name="psy", bufs=3, space="PSUM"))

        # x loads issued first on the sync queue (bufs=4: no slot waits, so
        # the DMA ring streams all four m-tiles back to back)
        x_tiles = []
        for m in range(N_MTILES):
            x_sb = x_pool.tile([M_TILE, IN], f32, tag="x")
            for h in range(2):
                nc.sync.dma_start(
                    out=x_sb[:, h * (IN // 2) : (h + 1) * (IN // 2)],
                    in_=x_ext[
                        m * M_TILE : (m + 1) * M_TILE,
                        h * (IN // 2) : (h + 1) * (IN // 2),
                    ],
                )
            x_tiles.append(x_sb)

        # ------------- prologue ---------------------------------------------
        identity = const.tile([128, 128], bf16)
        make_identity(nc, identity[:])

        # selection matrices first on gpsimd (no input deps):
        # S3[k, p]=1 iff k==p%16 ; S2h[k, p]=1 iff k==8h+p//16
        S3 = const.tile([D, 128], bf16)
        nc.gpsimd.memset(S3[:], 0.0)
        nc.gpsimd.affine_select(
            out=S3[:], in_=S3[:], compare_op=mybir.AluOpType.not_equal,
            fill=1.0, base=0, pattern=[[0, 8], [-1, 16]], channel_multiplier=1,
        )
        S2 = []
        for h in range(2):
            s = const.tile([D, 128], bf16, tag=f"S2_{h}")
            nc.gpsimd.memset(s[:], 0.0)
            nc.gpsimd.affine_select(
                out=s[:], in_=s[:], compare_op=mybir.AluOpType.not_equal,
                fill=1.0, base=-8 * h, pattern=[[-1, 8], [0, 16]],
                channel_multiplier=1,
            )
            S2.append(s)
        t_aug = []
        for i in range(2):
            t = const.tile([R + 1, M_TILE], bf16, tag=f"t_aug{i}")
            nc.gpsimd.memset(t[R : R + 1, :], 1.0)
            t_aug.append(t)

        # tiny loads (all on the scalar HWDGE queue: sync carries only x/y)
        U2n = const.tile([D, R], f32)
        U3n = const.tile([D, R], f32)
        nc.scalar.dma_start(out=U2n[:], in_=U2_ext[:])
        nc.scalar.dma_start(out=U3n[:], in_=U3_ext[:])
        U1rep = const.tile([128, D * R], f32)
        nc.scalar.dma_start(
            out=U1rep[:],
            in_=U1_ext[:].flatten().unsqueeze(0).broadcast_to([128, D * R]),
        )
        V1T = const.tile([R, D], f32)
        V2T = const.tile([R, D], f32)
        V3T = const.tile([R, D], f32)
        nc.scalar.dma_start(out=V1T[:], in_=V1_ext[:].transpose([1, 0]))
        nc.scalar.dma_start(out=V2T[:], in_=V2_ext[:].transpose([1, 0]))
        nc.scalar.dma_start(out=V3T[:], in_=V3_ext[:].transpose([1, 0]))
        lamT = const.tile([R, 1], f32)
        nc.scalar.dma_start(out=lamT[:], in_=lam_ext[:].unsqueeze(1))
        bias_sb = const.tile([1, OUT], f32)
        nc.scalar.dma_start(out=bias_sb[:], in_=bias_ext[:].unsqueeze(0))

        # bf16 casts of U2/U3 for the one-hot matmuls
        U2b = const.tile([D, R], bf16)
        U3b = const.tile([D, R], bf16)
        nc.vector.tensor_copy(U2b[:], U2n[:])
        nc.vector.tensor_copy(U3b[:], U3n[:])

        # replicate across partitions: psum rows p get U*[f(p), :]
        ps_rep = psx_pool.tile([128, 4 * R], f32, tag="ps_x")
        nc.tensor.matmul(ps_rep[:, 0:R], S3[:], U3b[:], start=True, stop=True)
        for h in range(2):
            nc.tensor.matmul(
                ps_rep[:, (1 + h) * R : (2 + h) * R], S2[h][:], U2b[:],
                start=True, stop=True,
            )
        U3rep = const.tile([128, R], f32)
        nc.vector.tensor_copy(U3rep[:], ps_rep[:, 0:R])
        # B23[p, 64h + r] = U2[8h + p//16, r] * U3[p%16, r]
        B23 = const.tile([128, 2 * R], f32)
        nc.vector.tensor_mul(
            B23[:].rearrange("p (h r) -> p h r", h=2),
            ps_rep[:, R : 3 * R].rearrange("p (h r) -> p h r", h=2),
            U3rep[:].unsqueeze(1).broadcast_to([128, 2, R]),
        )
        # A chunks: A_sb[p, 64c + r] = U1[c//2, r] * B23[p, 64*(c%2) + r]
        A_sb = const.tile([128, N_KCHUNKS * R], bf16)
        nc.vector.tensor_mul(
            A_sb[:].rearrange("p (i g r) -> p i g r", i=16, g=2),
            U1rep[:].rearrange("p (i r) -> p i r", i=16)
            .unsqueeze(2)
            .broadcast_to([128, 16, 2, R]),
            B23[:].rearrange("p (g r) -> p g r", g=2)
            .unsqueeze(1)
            .broadcast_to([128, 16, 2, R]),
        )

        # BT_aug rows 0..63: lam[r]*V1[o1,r]*V2[o2,r]*V3[o3,r]; row 64: bias
        V1Ts = const.tile([R, D], f32)
        nc.gpsimd.tensor_mul(V1Ts[:], V1T[:], lamT[:].broadcast_to([R, D]))
        W12v = const.tile([R, D * D], f32)
        nc.gpsimd.tensor_mul(
            W12v[:].rearrange("p (a b) -> p a b", a=16),
            V1Ts[:].unsqueeze(2).broadcast_to([R, D, D]),
            V2T[:].unsqueeze(1).broadcast_to([R, D, D]),
        )
        BT_aug = const.tile([R + 1, OUT], bf16)
        # first half gpsimd, second half DVE (after the A chain)
        nc.gpsimd.tensor_mul(
            BT_aug[0:R, 0 : OUT // 2].rearrange("p (w o) -> p w o", o=16),
            W12v[:, 0 : D * D // 2].unsqueeze(2).broadcast_to([R, D * D // 2, D]),
            V3T[:].unsqueeze(1).broadcast_to([R, D * D // 2, D]),
        )
        nc.vector.tensor_mul(
            BT_aug[0:R, OUT // 2 :].rearrange("p (w o) -> p w o", o=16),
            W12v[:, D * D // 2 :].unsqueeze(2).broadcast_to([R, D * D // 2, D]),
            V3T[:].unsqueeze(1).broadcast_to([R, D * D // 2, D]),
        )
        nc.scalar.copy(BT_aug[R : R + 1, :], bias_sb[:])

        # single persistent accumulation bank: cols (m%2)*128 per m-tile
        ps_t_all = pst_pool.tile([R, 2 * M_TILE], f32)

        # ---------------- main loop ------------------------------------------
        for m in range(N_MTILES):
            x_sb = x_tiles[m]
            x_bf = xbf_pool.tile([M_TILE, IN], bf16)

            ps_t = ps_t_all[:, m % 2 * M_TILE : (m % 2 + 1) * M_TILE]
            for g in range(N_GROUPS):
                lo, hi = g * GCHUNKS * KCHUNK, (g + 1) * GCHUNKS * KCHUNK
                if g % 2 == 0:
                    nc.scalar.copy(x_bf[:, lo:hi], x_sb[:, lo:hi])
                else:
                    nc.vector.tensor_copy(x_bf[:, lo:hi], x_sb[:, lo:hi])
                ps_x = psx_pool.tile([128, GCHUNKS * KCHUNK], f32, tag="ps_x")
                for j in range(GCHUNKS):
                    c = GCHUNKS * g + j
                    nc.tensor.matmul(
                        ps_x[:, j * KCHUNK : (j + 1) * KCHUNK],
                        x_bf[:, c * KCHUNK : (c + 1) * KCHUNK],
                        identity[:],
                        start=True,
                        stop=True,
                    )
                xT_sb = xT_pool.tile([128, GCHUNKS * KCHUNK], bf16)
                if g % 2 == 0:
                    nc.vector.tensor_copy(xT_sb[:], ps_x[:])
                else:
                    nc.scalar.copy(xT_sb[:], ps_x[:])
                for j in range(GCHUNKS):
                    c = GCHUNKS * g + j
                    nc.tensor.matmul(
                        ps_t,
                        A_sb[:, c * R : (c + 1) * R],
                        xT_sb[:, j * KCHUNK : (j + 1) * KCHUNK],
                        start=(c == 0),
                        stop=(c == N_KCHUNKS - 1),
                    )

            tt = t_aug[m % 2]
            nc.vector.tensor_copy(tt[0:R, :], ps_t)

            for n in range(8):
                ps_y = psy_pool.tile([M_TILE, 512], f32)
                nc.tensor.matmul(
                    ps_y[:],
                    tt[:],
                    BT_aug[:, n * 512 : (n + 1) * 512],
                    start=True,
                    stop=True,
                )
                y_sb = y_pool.tile([M_TILE, 512], f32)
                if n % 2 == 0:
                    nc.vector.tensor_copy(y_sb[:], ps_y[:])
                else:
                    nc.scalar.copy(y_sb[:], ps_y[:])
                dma_eng = nc.sync if n % 2 == 0 else nc.gpsimd
                dma_eng.dma_start(
                    out=out_ext[
                        m * M_TILE : (m + 1) * M_TILE, n * 512 : (n + 1) * 512
                    ],
                    in_=y_sb[:],
                )

    nc.compile()
    return nc


def _get_nc():
    if "nc" not in _CACHE:
        _CACHE["nc"] = _build_nc()
    return _CACHE["nc"]


def kernel(x, U1, U2, U3, V1, V2, V3, lam, bias):
    from concourse.bass_utils import run_bass_kernel_spmd

    nc = _get_nc()

    x = np.ascontiguousarray(np.asarray(x, dtype=np.float32))
    small = {
        "U1": U1, "U2": U2, "U3": U3,
        "V1": V1, "V2": V2, "V3": V3,
        "lam": lam, "bias": bias,
    }
    small = {
        k: np.ascontiguousarray(np.asarray(v, dtype=np.float32))
        for k, v in small.items()
    }

    in_maps = [
        {"x": x[i * B_SHARD : (i + 1) * B_SHARD], **small} for i in range(NCORES)
    ]
    res = run_bass_kernel_spmd(nc, in_maps, core_ids=list(range(NCORES)))
    _CACHE["last_results"] = res
    out = np.concatenate(
        [np.asarray(res.results[i]["out"]) for i in range(NCORES)], axis=0
    )
    return out.astype(np.float32)


def last_exec_time_ns():
    res = _CACHE.get("last_results")
    return None if res is None else res.exec_time_ns
